# revision 34
# baseline (speedup 1.0000x reference)
"""Trainium2 Bass kernel for GQA attention prefill (B=1, S=2048, D=4096,
32 Q heads / 8 KV heads, HD=128, RoPE, causal-masked softmax, output proj).

Sharding: tensor-parallel over heads across 8 NeuronCores. Core c computes
Q heads 4c..4c+3 with KV head c, then its partial contribution
attn_heads_c @ wo[rows of those heads]; the host sums the 8 partials
(the "all-reduce" after wo).

All matmuls run in bf16 (same PE rate as fp32r, half the SBUF/DMA traffic,
enables fast weight load); PSUM accumulation stays fp32. Verified on host:
end-to-end bf16 pipeline rel err ~4e-3 vs the 2e-2 gate. Everything
on-chip is kept in "transposed" layout [feature, seq] so Q/K projections,
scores, PV and the wo matmul all contract along the partition dim with no
transposes, except V which is transposed to [seq, hd] via PE-transposes.

DMA: all tensors are host-re-laid-out so every transfer is a single large
(1-8MB) descriptor: x streams in 1MB chunks (8 di-tiles) double-buffered
on the sync ring, weights (wq/wk/wv/wo) are one DMA each on the scalar
ring, mask patterns prefetch on the gpsimd ring, and the output leaves as
16 2MB strips on the sync ring.

RoPE: wq/wk columns are permuted on the host so rotary pairs (2i, 2i+1)
land on partitions (i, i+64); RoPE is then 2 half-partition copies + 3
full-width DVE ops per [128, 512] tile against host-prepared cos/sin.

Softmax: scores are computed transposed [k, q]; no max-subtraction (logits
are O(10); exp is safe in fp32). Sum over k (= partition dim) rides on an
all-ones [128,128] matmul accumulated alongside PV, which also broadcasts
the sums to all partitions. 1/sum runs on the DVE (reciprocal_approx_fast,
~18 bits) so the ACT engine stays a pure-Exp stream with zero activation-
table reloads.

wo: kept fully resident in SBUF (4MB bf16); chunks are emitted s-outer /
n-inner so each 128-row output block accumulates into a [128, 4096] strip
written out as one 2MB DMA. wo-chunks for q-block Q-1 interleave as PE
filler between attention tiles of q-block Q.
"""

import os
import sys
import types
from contextlib import ExitStack

import numpy as np
import ml_dtypes

BF = ml_dtypes.bfloat16

for _p in ("/opt/trn_rl_repo",):
    if _p not in sys.path:
        sys.path.insert(0, _p)


def _install_ntff_hook():
    """Best-effort registration of the axon NTFF profiling hook so that
    run_bass_kernel_spmd(trace=True) / BASS_TRACE=1 can report HW exec time.
    Harmless no-op if anything is missing."""
    try:
        import antenv

        if getattr(antenv, "axon_hooks", None) is not None:
            return
        mod = types.ModuleType("antenv.axon_hooks")
        holder = {}
        mod.set_axon_ntff_profile_hook = lambda h: holder.__setitem__("h", h)
        mod.get_axon_ntff_profile_hook = lambda: holder.get("h")
        sys.modules["antenv.axon_hooks"] = mod
        antenv.axon_hooks = mod
        from trn_agent_boot.trn_boot import _ntff_profile_via_ctypes

        h = _ntff_profile_via_ctypes("/opt/axon/libaxon_pjrt.so")
        if h is not None:
            mod.set_axon_ntff_profile_hook(h)
    except Exception:
        pass


_install_ntff_hook()

import concourse.bass as bass  # noqa: E402
import concourse.tile as tile  # noqa: E402
from concourse import bacc, mybir  # noqa: E402
from concourse import bass_utils  # noqa: E402

F32 = mybir.dt.float32
BF16 = mybir.dt.bfloat16
EXP = mybir.ActivationFunctionType.Exp

NCORES = 8
D = 4096
NH, NKV, HD = 32, 8, 128
HPC = NH // NCORES  # 4 query heads per core
SCALE = float(HD) ** -0.5
NEG = -1e9
SB = 512  # seq block (matmul free dim)

_PROG_CACHE: dict = {}
_HOST_CACHE: dict = {}
LAST_RESULTS = None  # BassKernelResults of the most recent run (for test.py)

# Matmul datapath dtype: "bf16" (half SBUF/DMA, wo resident, FWL) or
# "f32r" (TF32-like fp32; same PE rate, double traffic, wo streamed).
MM_DTYPE = os.environ.get("BASS_MM_DTYPE", "bf16")


def _build(S: int, mask_mode: str, mmdt: str):
    """Emit + compile the per-core Bass program. mask_mode: none|causal|general."""
    NB = S // SB        # seq blocks of 512
    DT = D // 128       # contraction tiles for projections
    KT = S // 128       # k tiles
    DIAG = SB // 128    # k-tiles crossing the diagonal per q block (4)
    BF16MM = mmdt == "bf16"
    MM = BF16 if BF16MM else mybir.dt.float32r
    CH = 8 if BF16MM else 4   # x di-tiles per DMA chunk (1MB either way)
    NCH = DT // CH      # x chunks per seq block
    WO_RES = BF16MM     # wo resident in SBUF (4MB bf16) vs streamed per-Q

    nc = bacc.Bacc("TRN2", target_bir_lowering=False, debug=False,
                   num_devices=NCORES)

    def din(name, shape, dt=None):
        mm = dt is None
        dt = dt or (BF16 if BF16MM else F32)
        t = nc.dram_tensor(name, shape, dt, kind="ExternalInput").ap()
        if mm and not BF16MM:
            t = t.bitcast(mybir.dt.float32r)
        return t

    xtb = din("xtb", [NB, NCH, 128, CH, SB])      # pre-tiled x.T, bf16
    wq3 = din("wq3", [128, DT, HPC * HD])         # rope-permuted columns
    wk3 = din("wk3", [128, DT, HD])               # rope-permuted columns
    wv3 = din("wv3", [128, DT, HD])
    wo5 = din("wo5", [128, (D // SB) * HPC * SB])  # resident wo, n-major cols
    cos2 = din("cos2", [128, S], F32)  # rows j and j+64 = cos(ang[:, j])
    sin2 = din("sin2", [128, S], F32)  # row j = -sin, row j+64 = +sin
    ident = din("ident", [128, 128])
    ones = din("ones", [128, 128])
    if mask_mode == "causal":
        pats = din("pats", [128, DIAG * SB], F32)
    if mask_mode == "general":
        maskt4 = din("maskt4", [NB, 128, KT, SB], F32)  # pre-tiled mask.T*sqrt(HD)
    out5 = nc.dram_tensor("o5", [S // 128, 128, D], F32,
                          kind="ExternalOutput").ap()

    with tile.TileContext(nc) as tc, ExitStack() as ctx:
        # ---- persistent activations (live through all phases) ----
        apool = ctx.enter_context(tc.tile_pool(name="acts", bufs=1))
        xq_sb = apool.tile([128, HPC * S], MM, tag="xq")  # per-head [hd, s]
        xk_sb = apool.tile([128, S], MM, tag="xk")
        v_sb = apool.tile([128, S], MM, tag="v")          # [s%128, hd] tiles
        if WO_RES:
            wo_sb = apool.tile([128, (D // SB) * HPC * SB], MM, tag="wo")
        ones_sb = apool.tile([128, 128], MM, tag="ones")
        if mask_mode == "causal":
            pats_sb = apool.tile([128, DIAG * SB], F32, tag="pats")

        # prefetch phase-B constants on the gpsimd ring, weights on the
        # scalar ring; x chunks stream on the sync ring below.
        nc.gpsimd.dma_start(ones_sb[:], ones)
        if mask_mode == "causal":
            nc.gpsimd.dma_start(pats_sb[:], pats)

        # ================= Phase A: projections + RoPE + V transpose ======
        with tc.tile_pool(name="wproj", bufs=1) as wpool, \
             tc.tile_pool(name="aconst", bufs=1) as acpool, \
             tc.tile_pool(name="xin", bufs=3 if BF16MM else 2) as xpool, \
             tc.tile_pool(name="ptmp", bufs=2) as tpool, \
             tc.tile_pool(name="rtmp", bufs=5 if BF16MM else 2) as rpool, \
             tc.tile_pool(name="pjps", bufs=1, space="PSUM") as pjps, \
             tc.tile_pool(name="vtps", bufs=1, space="PSUM") as vtps:
            wq_sb = wpool.tile([128, DT * HPC * HD], MM, tag="wq")
            wk_sb = wpool.tile([128, DT * HD], MM, tag="wk")
            wv_sb = wpool.tile([128, DT * HD], MM, tag="wv")
            cos_sb = acpool.tile([128, S], F32, tag="cos")
            sin_sb = acpool.tile([128, S], F32, tag="sin")
            id_sb = acpool.tile([128, 128], MM, tag="id")
            # scalar (ACT HWDGE) ring: weights in first-use order; wq split
            # so pq matmuls of chunk 0 wait only on its first quarter.
            nc.scalar.dma_start(
                wk_sb[:].rearrange("p (dt c) -> p dt c", c=HD), wk3)
            nc.scalar.dma_start(
                wv_sb[:].rearrange("p (dt c) -> p dt c", c=HD), wv3)
            QW = DT // 4
            for qw in range(4):
                nc.scalar.dma_start(
                    wq_sb[:, qw * QW * HPC * HD:(qw + 1) * QW * HPC * HD]
                    .rearrange("p (dt c) -> p dt c", c=HPC * HD),
                    wq3[:, qw * QW:(qw + 1) * QW])
            nc.scalar.dma_start(cos_sb[:], cos2)
            nc.scalar.dma_start(sin_sb[:], sin2)
            # gpsimd (SWDGE) ring: wo + small constants (needed late)
            nc.gpsimd.dma_start(id_sb[:], ident)
            if WO_RES:
                nc.gpsimd.dma_start(wo_sb[:], wo5)

            def rope_read(ps, b):
                """Pass 1: the ops that READ the psum (frees the bank)."""
                cs = cos_sb[:, b * SB:(b + 1) * SB]
                t2 = rpool.tile([128, SB], F32, tag="t2")
                nc.vector.tensor_mul(t2[:], ps[:], cs)
                swp = rpool.tile([128, SB], F32, tag="swp")
                nc.scalar.copy(swp[0:64, :], ps[64:128, :])
                nc.scalar.copy(swp[64:128, :], ps[0:64, :])
                return t2, swp

            def rope_write(t2, swp, dst, b):
                """Pass 2: finish RoPE into dst (bf16)."""
                sn = sin_sb[:, b * SB:(b + 1) * SB]
                t1 = tpool.tile([128, SB], F32, tag="t1")
                nc.vector.tensor_mul(t1[:], swp[:], sn)
                nc.vector.tensor_add(dst, t1[:], t2[:])

            for b in range(NB):
                pq = [pjps.tile([128, SB], F32, tag=f"pq{h}", name=f"pq{h}",
                                bufs=2 if h == 0 else 1)
                      for h in range(HPC)]
                pk = pjps.tile([128, SB], F32, tag="pk")
                pv = pjps.tile([128, SB], F32, tag="pv")
                for c in range(NCH):
                    xc = xpool.tile([128, CH * SB], MM, tag="xc")
                    nc.sync.dma_start(
                        xc[:].rearrange("p (k j) -> p k j", j=SB), xtb[b, c])
                    # k/v matmuls for the whole chunk first: their ropes
                    # finish earliest, so the next block's pk/pv (head of
                    # the PE queue) never stall on a trailing q-rope.
                    for k in range(CH):
                        di = c * CH + k
                        xt_t = xc[:, k * SB:(k + 1) * SB]
                        st, sp = (di == 0), (di == DT - 1)
                        nc.tensor.matmul(pk[:], wk_sb[:, di * HD:(di + 1) * HD],
                                         xt_t, start=st, stop=sp)
                        nc.tensor.matmul(pv[:], wv_sb[:, di * HD:(di + 1) * HD],
                                         xt_t, start=st, stop=sp)
                    for k in range(CH):
                        di = c * CH + k
                        xt_t = xc[:, k * SB:(k + 1) * SB]
                        st, sp = (di == 0), (di == DT - 1)
                        for h in range(HPC):
                            nc.tensor.matmul(
                                pq[h][:],
                                wq_sb[:, di * HPC * HD + h * HD:
                                      di * HPC * HD + (h + 1) * HD],
                                xt_t, start=st, stop=sp)
                # Two-pass RoPE (bf16: rtmp has 5 bufs): emit every
                # psum-reading op first so all six PSUM banks release within
                # ~4us, then the finishing ops. f32r (2 bufs): adjacent
                # read/write per rope, k/v first.
                rk = rope_read(pk, b)
                # V: [hd, s] psum -> natural [s, hd] via 4 PE transposes
                vt = tpool.tile([128, SB], MM, tag="vt")
                nc.scalar.copy(vt[:], pv[:])
                if not BF16MM:
                    rope_write(*rk, xk_sb[:, b * SB:(b + 1) * SB], b)
                    rq = []
                else:
                    rq = [rope_read(pq[h], b) for h in range(HPC)]
                    rope_write(*rk, xk_sb[:, b * SB:(b + 1) * SB], b)
                pvn = vtps.tile([128, SB], MM, tag="pvn")
                for j in range(SB // 128):
                    nc.tensor.transpose(pvn[:, j * 128:(j + 1) * 128],
                                        vt[:, j * 128:(j + 1) * 128], id_sb[:])
                nc.scalar.copy(v_sb[:, b * SB:(b + 1) * SB], pvn[:])
                for h in range(HPC):
                    dst = xq_sb[:, h * S + b * SB: h * S + (b + 1) * SB]
                    if BF16MM:
                        rope_write(*rq[h], dst, b)
                    else:
                        rope_write(*rope_read(pq[h], b), dst, b)

        # ========== Phases B+C: attention + output proj, pipelined =======
        # Flat software-pipelined loop over attention tiles (h, Q, t):
        # producer P(i) = scores matmul -> mask add (DVE) -> exp (ACT);
        # consumer K(i) = sums matmul + PV matmul, emitted LOOK tiles later
        # so the exp latency stays off PE's critical path. The wo-projection
        # matmul chunks for q-block Q-1 are interleaved as PE filler, which
        # keeps PE busy while ACT churns exps. Group tails compute 1/sum on
        # the DVE (reciprocal_approx_fast) so ACT never reloads its table.
        LOOK = 3
        at_pool = ctx.enter_context(tc.tile_pool(name="attn", bufs=1))
        at_sb = at_pool.tile([128, HPC * S], MM, tag="at")
        with tc.tile_pool(name="probs", bufs=8) as ppool, \
             tc.tile_pool(name="btmp", bufs=3) as btpool, \
             tc.tile_pool(name="osb", bufs=2) as opool, \
             tc.tile_pool(name="woc", bufs=2) as wop, \
             tc.tile_pool(name="sps", bufs=3, space="PSUM") as sps, \
             tc.tile_pool(name="atps", bufs=2, space="PSUM") as atps, \
             tc.tile_pool(name="smps", bufs=1, space="PSUM") as smps, \
             tc.tile_pool(name="ops", bufs=2, space="PSUM") as ops, \
             ExitStack() as bctx:
            if mask_mode == "general":
                mpool = bctx.enter_context(tc.tile_pool(name="mstrip", bufs=1))

            def wo_chunk_mms(po, s, wsrc):
                for h in range(HPC):
                    nc.tensor.matmul(
                        po[:],
                        at_sb[:, h * S + s * 128: h * S + (s + 1) * 128],
                        wsrc[:, h * SB:(h + 1) * SB],
                        start=(h == 0), stop=(h == HPC - 1))

            def wo_fill(Qc):
                """Generator of phase-C chunk emitters for q-block Qc."""
                if WO_RES:
                    # s-outer / n-inner: each 128-row output block accumulates
                    # into one [128, D] strip, stored as a single 2MB DMA.
                    for s in range(Qc * DIAG, (Qc + 1) * DIAG):
                        ot = opool.tile([128, D], F32, tag="ot", name="ot")
                        for n in range(D // SB):
                            def chunk(n=n, s=s, ot=ot):
                                po = ops.tile([128, SB], F32, tag="po",
                                              name="po")
                                wo_chunk_mms(
                                    po, s, wo_sb[:, (n * HPC) * SB:
                                                 (n * HPC + HPC) * SB])
                                if n % 2 == 0:
                                    nc.vector.tensor_copy(
                                        ot[:, n * SB:(n + 1) * SB], po[:])
                                else:
                                    nc.scalar.copy(
                                        ot[:, n * SB:(n + 1) * SB], po[:])
                                if n == D // SB - 1:
                                    nc.sync.dma_start(out5[s], ot[:])
                            yield chunk
                else:
                    # n-outer / s-inner: stream 1MB wo chunks, store each
                    # [128, SB] result directly.
                    for n in range(D // SB):
                        woc = wop.tile([128, HPC * SB], MM, tag="woc",
                                       name="woc")
                        nc.gpsimd.dma_start(
                            woc[:], wo5[:, (n * HPC) * SB:(n * HPC + HPC) * SB])
                        for s in range(Qc * DIAG, (Qc + 1) * DIAG):
                            def chunk(n=n, s=s, woc=woc):
                                po = ops.tile([128, SB], F32, tag="po",
                                              name="po")
                                wo_chunk_mms(po, s, woc[:])
                                ot = opool.tile([128, SB], F32, tag="ot",
                                                name="ot", bufs=4)
                                if n % 2 == 0:
                                    nc.vector.tensor_copy(ot[:], po[:])
                                else:
                                    nc.scalar.copy(ot[:], po[:])
                                nc.sync.dma_start(
                                    out5[s][:, n * SB:(n + 1) * SB], ot[:])
                            yield chunk

            state = {}  # i -> (pr, pat_, psm, h, Q, t, kmax)

            def produce(i, h, Q, t, kmax, mstrip):
                qs = xq_sb[:, h * S + Q * SB: h * S + (Q + 1) * SB]
                pss = sps.tile([128, SB], F32, tag="pss", name="pss")
                nc.tensor.matmul(pss[:], xk_sb[:, t * 128:(t + 1) * 128],
                                 qs, start=True, stop=True)
                if mask_mode == "causal" and t >= kmax - DIAG:
                    m = t - (kmax - DIAG)
                    nc.vector.tensor_add(pss[:], pss[:],
                                         pats_sb[:, m * SB:(m + 1) * SB])
                elif mask_mode == "general":
                    nc.vector.tensor_add(pss[:], pss[:],
                                         mstrip[:, t * SB:(t + 1) * SB])
                pr = ppool.tile([128, SB], BF16, tag="pr", name="pr")
                nc.scalar.activation(pr[:], pss[:], EXP, scale=SCALE)
                if t == 0:
                    pat_ = atps.tile([128, SB], F32, tag="pat", name="pat")
                    psm = smps.tile([128, SB], F32, tag="psm", name="psm")
                else:
                    _, pat_, psm = state[i - 1][:3]
                state[i] = (pr, pat_, psm, h, Q, t, kmax)

            def consume(i):
                pr, pat_, psm, h, Q, t, kmax = state.pop(i)
                nc.tensor.matmul(psm[:], ones_sb[:], pr[:],
                                 start=(t == 0), stop=(t == kmax - 1))
                nc.tensor.matmul(pat_[:], v_sb[:, t * 128:(t + 1) * 128],
                                 pr[:], start=(t == 0), stop=(t == kmax - 1))
                if t == kmax - 1:
                    # 1/s on DVE (~18-bit accurate, single op); keeps the
                    # ACT engine a pure-Exp stream (no table reloads).
                    rcp = btpool.tile([128, SB], F32, tag="rcp", name="rcp")
                    nc.vector.reciprocal_approx_fast(rcp[:], psm[:])
                    nc.vector.tensor_mul(
                        at_sb[:, h * S + Q * SB: h * S + (Q + 1) * SB],
                        pat_[:], rcp[:])

            for Q in range(NB):
                kmax = DIAG * (Q + 1) if mask_mode == "causal" else KT
                mstrip = None
                if mask_mode == "general":
                    mstrip = mpool.tile([128, KT * SB], F32, tag="ms", name="ms")
                    nc.gpsimd.dma_start(
                        mstrip[:].rearrange("p (t j) -> p t j", j=SB),
                        maskt4[Q])
                tiles = [(h, t) for h in range(HPC) for t in range(kmax)]
                fillers = list(wo_fill(Q - 1)) if Q > 0 else []
                nf, nt = len(fillers), len(tiles)
                fdone = 0
                base = Q * 10000
                for i, (h, t) in enumerate(tiles):
                    produce(base + i, h, Q, t, kmax, mstrip)
                    while fdone * nt < nf * (i + 1):
                        fillers[fdone]()
                        fdone += 1
                    if i >= LOOK:
                        consume(base + i - LOOK)
                for i in range(nt - LOOK, nt):
                    consume(base + i)
                while fdone < nf:
                    fillers[fdone]()
                    fdone += 1
            for f in wo_fill(NB - 1):
                f()

    nc.compile()
    return nc


def _get_prog(S: int, mask_mode: str):
    key = (S, mask_mode, MM_DTYPE)
    if key not in _PROG_CACHE:
        _PROG_CACHE[key] = _build(S, mask_mode, MM_DTYPE)
    return _PROG_CACHE[key]


def _mask_mode(mask: np.ndarray) -> str:
    S = mask.shape[0]
    if not mask.any():
        return "none"
    causal = np.triu(np.full((S, S), np.float32(NEG), dtype=np.float32), k=1)
    if np.array_equal(mask, causal):
        return "causal"
    return "general"


def _prep_host(x, wq, wk, wv, wo, freqs_cos, freqs_sin, mode):
    """Host-side re-layouts (hashable on id of the input arrays)."""
    MD = BF if MM_DTYPE == "bf16" else np.float32
    CHm = 8 if MM_DTYPE == "bf16" else 4
    S = x.shape[1]
    DT, NB, NCH = D // 128, S // SB, (D // 128) // CHm
    x2 = np.ascontiguousarray(x[0])                 # [S, D]
    xt = x2.T.astype(MD)                            # [D, S]
    # [NB, NCH, 128, CH, SB]: per (b, chunk), partition-first contiguous
    xtb = np.ascontiguousarray(
        xt.reshape(NCH, CHm, 128, NB, SB).transpose(3, 0, 2, 1, 4))
    perm = np.concatenate([np.arange(0, HD, 2), np.arange(1, HD, 2)])
    cosT = np.ascontiguousarray(np.asarray(freqs_cos, np.float32).T)  # [64, S]
    sinT = np.ascontiguousarray(np.asarray(freqs_sin, np.float32).T)
    cos2 = np.concatenate([cosT, cosT], axis=0)     # [128, S]
    sin2 = np.concatenate([-sinT, sinT], axis=0)
    ident = np.eye(128, dtype=MD)
    onesm = np.ones((128, 128), dtype=MD)

    common = {"xtb": xtb, "cos2": cos2, "sin2": sin2, "ident": ident,
              "ones": onesm}
    if mode == "causal":
        DIAG = SB // 128
        i = np.arange(128)[:, None]
        j = np.arange(SB)[None, :]
        common["pats"] = np.ascontiguousarray(np.concatenate(
            [np.where(128 * m + i > j, np.float32(NEG), np.float32(0.0))
             for m in range(DIAG)], axis=1)).astype(np.float32)  # [128, 4*SB]

    in_maps = []
    for c in range(NCORES):
        hs = slice(c * HPC * HD, (c + 1) * HPC * HD)
        wq_c = wq[:, hs].reshape(D, HPC, HD)[:, :, perm].reshape(D, HPC * HD)
        wk_c = wk[:, c * HD:(c + 1) * HD][:, perm]
        wo_c = wo[hs, :]
        # wo5: [128 (hd), n, h, SB] flattened -> n-major columns
        wo5 = np.ascontiguousarray(
            wo_c.reshape(HPC, 128, D // SB, SB).transpose(1, 2, 0, 3)
        ).reshape(128, (D // SB) * HPC * SB).astype(MD)
        in_maps.append(dict(
            common,
            wq3=np.ascontiguousarray(
                wq_c.reshape(DT, 128, HPC * HD).transpose(1, 0, 2)).astype(MD),
            wk3=np.ascontiguousarray(
                wk_c.reshape(DT, 128, HD).transpose(1, 0, 2)).astype(MD),
            wv3=np.ascontiguousarray(
                wv[:, c * HD:(c + 1) * HD].reshape(DT, 128, HD)
                .transpose(1, 0, 2)).astype(MD),
            wo5=wo5,
        ))
    return in_maps


def kernel(x, wq, wk, wv, wo, freqs_cos, freqs_sin, positions, mask):
    x = np.asarray(x, dtype=np.float32)
    B = x.shape[0]
    assert B == 1
    S = x.shape[1]
    mask = np.asarray(mask, dtype=np.float32)
    mode = _mask_mode(mask)
    nc = _get_prog(S, mode)

    wq = np.asarray(wq, np.float32)
    wk = np.asarray(wk, np.float32)
    wv = np.asarray(wv, np.float32)
    wo = np.asarray(wo, np.float32)
    hkey = (id(x), id(wq), id(wo), mode, S, MM_DTYPE)
    if _HOST_CACHE.get("key") != hkey:
        in_maps = _prep_host(x, wq, wk, wv, wo, freqs_cos, freqs_sin, mode)
        if mode == "general":
            KT, NB = S // 128, S // SB
            mt = (mask.T * np.float32(np.sqrt(HD))).astype(np.float32)
            m4 = np.ascontiguousarray(
                mt.reshape(KT, 128, NB, SB).transpose(2, 1, 0, 3))
            for im in in_maps:
                im["maskt4"] = m4
        _HOST_CACHE["key"] = hkey
        _HOST_CACHE["maps"] = in_maps
    in_maps = _HOST_CACHE["maps"]

    global LAST_RESULTS
    trace = bool(os.environ.get("BASS_TRACE"))
    res = bass_utils.run_bass_kernel_spmd(
        nc, in_maps, core_ids=list(range(NCORES)), trace=trace)
    LAST_RESULTS = res
    acc = res.results[0]["o5"].astype(np.float32).copy()
    for c in range(1, NCORES):
        acc += res.results[c]["o5"]
    return acc.reshape(1, S, D)
